# revision 14
# baseline (speedup 1.0000x reference)
"""Trainium2 Bass kernel for nn_NnBoard768 (sparse embedding lookup NNUE head).

Strategy (data-parallel over batch, 8 cores):
  - Each core handles 1024 of the 8192 batch rows. Batch row b sits at
    SBUF partition b%128, free-slot b//128.
  - The feature table is shipped to HBM as bf16 (half the gather traffic;
    PSUM accumulation below stays fp32, so only the table entries round).
  - Rows are fetched with the TIE-accelerated `dma_gather` instruction.
    Its indices are int16, so each (side, k) gather runs as two passes:
    pass A covers table rows < 32767, pass B covers the rest (rebased).
    Out-of-range slots get index -1, which the DGE turns into a read of
    the row *before* the pass base; the shipped table therefore carries
    explicit zero guard rows at both bases (and the tail), making every
    invalid-slot read a zero row that adds nothing. The hardware also
    skips any trailing run of -1s entirely (leaving stale SBUF data), so
    the last index of every gather is forced valid, pointing at a guard
    row when the real row belongs to the other pass.
  - The sum over the 32 active features runs on the tensor engine:
    identity matmuls accumulate every gathered tile into PSUM (fp32).
  - Epilogue on DVE/ACT: +b_ft, clip(0,1), dot with W_out, +b_out, sigmoid.
"""

import sys

sys.path.insert(0, "/opt/trn_rl_repo")

import numpy as np
import ml_dtypes

from concourse import bacc, bass, mybir
from concourse.masks import make_identity
import concourse.tile as tile
from concourse.bass_utils import run_bass_kernel_spmd

P = 128          # SBUF partitions
K = 32           # nnz (active features per position)
J = 8            # batch slots per partition
F = 512          # feature-table output width
NCORES = 8
BPC = P * J      # batch rows per core (1024)
FT_IN = 40960
S2 = 32767       # rows < S2 -> pass A; rows >= S2 -> pass B
NB = FT_IN - S2  # 8193 pass-B rows
# device table layout: [guard0, W[0:S2], guard1, W[S2:], guard2]
VDEV = FT_IN + 3
AJUNK = S2       # pass-A index of guard1 (zero row) in the A view
BJUNK = NB       # pass-B index of guard2 (zero row) in the B view
S16 = BPC // 16  # columns of the 16-partition-wrapped index tile (64)

f32 = mybir.dt.float32
bf16 = mybir.dt.bfloat16
i16 = mybir.dt.int16
Alu = mybir.AluOpType

GBUFS = 6        # in-flight gather tiles per (A/B) tag


def _build(fast: bool):
    nc = bacc.Bacc("TRN2", target_bir_lowering=False, debug=False, num_devices=NCORES)

    idx_in = {}
    for side in ("stm", "nstm"):
        for part in ("a", "b"):
            idx_in[(side, part)] = nc.dram_tensor(
                f"i{part}_{side}", [P, K, S16], i16, kind="ExternalInput"
            )
    wft = nc.dram_tensor("w_ft", [VDEV, F], bf16, kind="ExternalInput")
    bft = nc.dram_tensor("bft", [P, F], f32, kind="ExternalInput")
    w1 = nc.dram_tensor("w1", [P, F], f32, kind="ExternalInput")
    w2 = nc.dram_tensor("w2", [P, F], f32, kind="ExternalInput")
    bout = nc.dram_tensor("bout", [P, 1], f32, kind="ExternalInput")
    if not fast:
        vals = nc.dram_tensor("vals", [P, K, J], f32, kind="ExternalInput")
    out = nc.dram_tensor("out", [P, J], f32, kind="ExternalOutput")

    gbufs = GBUFS if fast else 2
    with tile.TileContext(nc) as tc:
        with tc.tile_pool(name="sbuf", bufs=1) as pool, \
             tc.tile_pool(name="gather", bufs=gbufs) as gpool, \
             tc.tile_pool(name="psum", bufs=1, space="PSUM") as ppool:
            idx_sb = {}
            for side_i, side in enumerate(("stm", "nstm")):
                for part in ("a", "b"):
                    t = pool.tile(
                        [P, K, S16], i16,
                        tag=f"i{part}{side_i}", name=f"i{part}_{side}_sb",
                    )
                    nc.sync.dma_start(out=t[:], in_=idx_in[(side, part)][:])
                    idx_sb[(side_i, part)] = t
            bft_sb = pool.tile([P, F], f32, tag="bft", name="bft_sb")
            nc.sync.dma_start(out=bft_sb[:], in_=bft[:])
            w_sb = [
                pool.tile([P, F], f32, tag="w1", name="w1_sb"),
                pool.tile([P, F], f32, tag="w2", name="w2_sb"),
            ]
            nc.sync.dma_start(out=w_sb[0][:], in_=w1[:])
            nc.sync.dma_start(out=w_sb[1][:], in_=w2[:])
            bout_sb = pool.tile([P, 1], f32, tag="bout", name="bout_sb")
            nc.sync.dma_start(out=bout_sb[:], in_=bout[:])
            ident = pool.tile([P, P], bf16, tag="ident", name="ident")
            make_identity(nc, ident[:])
            if not fast:
                vals_sb = pool.tile([P, K, J], f32, tag="vals", name="vals_sb")
                nc.sync.dma_start(out=vals_sb[:], in_=vals[:])

            def bcast(t2d):  # [P, F] -> [P, J, F] AP (stride-0 over J)
                return t2d[:].rearrange("p (j f) -> p j f", j=1).broadcast_to([P, J, F])

            z = [
                pool.tile([P, J], f32, tag=f"z{side}", name=f"z{side}")
                for side in range(2)
            ]
            for side in range(2):
                if fast:
                    acc = ppool.tile(
                        [P, J, F], f32, space="PSUM", tag="acc", name=f"acc{side}"
                    )
                else:
                    acc = pool.tile([P, J, F], f32, tag=f"sacc{side}", name=f"sacc{side}")
                for k in range(K):
                    ga = gpool.tile([P, J, F], bf16, tag="ga", name="ga")
                    gb = gpool.tile([P, J, F], bf16, tag="gb", name="gb")
                    nc.gpsimd.dma_gather(
                        ga[:], wft[1:, :], idx_sb[(side, "a")][:, k, :],
                        num_idxs=BPC, num_idxs_reg=BPC, elem_size=F,
                    )
                    nc.gpsimd.dma_gather(
                        gb[:], wft[S2 + 2:, :], idx_sb[(side, "b")][:, k, :],
                        num_idxs=BPC, num_idxs_reg=BPC, elem_size=F,
                    )
                    if fast:
                        for j in range(J):
                            nc.tensor.matmul(
                                acc[:, j, :], ident[:], ga[:, j, :],
                                start=(k == 0), stop=False,
                            )
                        for j in range(J):
                            nc.tensor.matmul(
                                acc[:, j, :], ident[:], gb[:, j, :],
                                start=False, stop=(k == K - 1),
                            )
                    else:
                        vb = (
                            vals_sb[:, k, :]
                            .rearrange("p (j f) -> p j f", f=1)
                            .broadcast_to([P, J, F])
                        )
                        t = gpool.tile([P, J, F], f32, tag="t", name="t")
                        nc.vector.tensor_tensor(out=t[:], in0=ga[:], in1=gb[:], op=Alu.add)
                        if k == 0:
                            nc.vector.tensor_tensor(out=acc[:], in0=t[:], in1=vb, op=Alu.mult)
                        else:
                            nc.vector.tensor_tensor(out=t[:], in0=t[:], in1=vb, op=Alu.mult)
                            nc.vector.tensor_tensor(out=acc[:], in0=acc[:], in1=t[:], op=Alu.add)

                # epilogue: h = clip(acc + b_ft, 0, 1) * w_side; z = sum_f h
                h = pool.tile([P, J, F], f32, tag=f"h{side}", name=f"h{side}")
                nc.vector.tensor_tensor(out=h[:], in0=acc[:], in1=bcast(bft_sb), op=Alu.add)
                nc.vector.tensor_scalar(
                    out=h[:], in0=h[:], scalar1=0.0, scalar2=1.0,
                    op0=Alu.max, op1=Alu.min,
                )
                nc.vector.tensor_tensor(out=h[:], in0=h[:], in1=bcast(w_sb[side]), op=Alu.mult)
                nc.vector.tensor_reduce(
                    out=z[side][:], in_=h[:], axis=mybir.AxisListType.X, op=Alu.add
                )
            nc.vector.tensor_tensor(out=z[0][:], in0=z[0][:], in1=z[1][:], op=Alu.add)
            out_sb = pool.tile([P, J], f32, tag="out", name="out_sb")
            nc.scalar.activation(
                out=out_sb[:],
                in_=z[0][:],
                func=mybir.ActivationFunctionType.Sigmoid,
                bias=bout_sb[:, :1],
            )
            nc.sync.dma_start(out=out.ap(), in_=out_sb[:])

    nc.compile()
    return nc


_cache = {}


def _get(fast: bool):
    if fast not in _cache:
        _cache[fast] = _build(fast)
    return _cache[fast]


def _prep_table(W_ft: np.ndarray) -> np.ndarray:
    """f32 [40960, 512] -> bf16 [VDEV, 512] with zero guard rows around
    both pass bases and at the tail."""
    w = np.zeros((VDEV, F), dtype=ml_dtypes.bfloat16)
    w[1:S2 + 1] = W_ft[:S2].astype(ml_dtypes.bfloat16)
    w[S2 + 2:S2 + 2 + NB] = W_ft[S2:].astype(ml_dtypes.bfloat16)
    return w


def _prep_idx(idx_core: np.ndarray):
    """[1024, 32] int32 -> (A, B) int16 arrays of shape [128, 32, 64].

    Index g (= batch row b) for feature-slot k lives at partition g%16,
    column g//16 (replicated across the 8 16-partition groups).
    Out-of-pass slots get -1 (a read of the zero guard row before the
    pass base); the last index stays valid via the in-view guard rows.
    """
    t3 = idx_core.astype(np.int64).reshape(S16, 16, K).transpose(2, 1, 0)  # [K,16,S16]
    a = np.where(t3 < S2, t3, -1).astype(np.int16)
    b = np.where(t3 >= S2, t3 - S2, -1).astype(np.int16)
    # keep the final index (r=15, s=S16-1) valid in both passes
    last_a = a[:, 15, S16 - 1]
    last_b = b[:, 15, S16 - 1]
    a[:, 15, S16 - 1] = np.where(last_a < 0, AJUNK, last_a)
    b[:, 15, S16 - 1] = np.where(last_b < 0, BJUNK, last_b)
    a = np.ascontiguousarray(np.tile(a, (1, 8, 1)).transpose(1, 0, 2))  # [128,K,S16]
    b = np.ascontiguousarray(np.tile(b, (1, 8, 1)).transpose(1, 0, 2))
    return a, b


def kernel(stm_indices, nstm_indices, values, W_ft, b_ft, W_out, b_out, _trace=False):
    stm_indices = np.asarray(stm_indices)
    nstm_indices = np.asarray(nstm_indices)
    values = np.asarray(values, dtype=np.float32)
    W_ft = np.ascontiguousarray(np.asarray(W_ft, dtype=np.float32))
    b_ft = np.asarray(b_ft, dtype=np.float32)
    W_out = np.asarray(W_out, dtype=np.float32)
    b_out = np.asarray(b_out, dtype=np.float32)

    fast = bool(np.all(values == 1.0))
    nc = _get(fast)

    w_dev = _prep_table(W_ft)
    bft_rep = np.ascontiguousarray(np.broadcast_to(b_ft, (P, F)).astype(np.float32))
    w1_rep = np.ascontiguousarray(np.broadcast_to(W_out[:F, 0], (P, F)).astype(np.float32))
    w2_rep = np.ascontiguousarray(np.broadcast_to(W_out[F:, 0], (P, F)).astype(np.float32))
    bout_rep = np.full((P, 1), b_out[0], dtype=np.float32)

    in_maps = []
    for c in range(NCORES):
        sl = slice(c * BPC, (c + 1) * BPC)
        m = {
            "w_ft": w_dev,
            "bft": bft_rep,
            "w1": w1_rep,
            "w2": w2_rep,
            "bout": bout_rep,
        }
        for side, arr in (("stm", stm_indices), ("nstm", nstm_indices)):
            a, b = _prep_idx(arr[sl])
            m[f"ia_{side}"] = a
            m[f"ib_{side}"] = b
        if not fast:
            # vals[p, k, j] = values[j*128 + p, k]
            m["vals"] = np.ascontiguousarray(
                values[sl].reshape(J, P, K).transpose(1, 2, 0)
            )
        in_maps.append(m)

    res = run_bass_kernel_spmd(
        nc, in_maps, core_ids=list(range(NCORES)), trace=_trace
    )
    # out[p, j] holds batch row j*128 + p
    out = np.concatenate(
        [res.results[c]["out"].T.reshape(BPC) for c in range(NCORES)]
    ).reshape(8192, 1)
    if _trace:
        return out, res
    return out


# revision 15
# speedup vs baseline: 3.6127x; 3.6127x over previous
"""Trainium2 Bass kernel for nn_NnBoard768 (sparse embedding lookup NNUE head).

Strategy (data-parallel over batch, 8 cores):
  - Each core handles 1024 of the 8192 batch rows. Batch row b sits at
    SBUF partition b%128, free-slot b//128.
  - The feature table is shipped to HBM as bf16 (half the gather traffic;
    PSUM accumulation below stays fp32, so only the table entries round).
  - Rows are fetched with the TIE-accelerated `dma_gather` instruction.
    Its indices are int16, so each (side, k) gather runs as two passes:
    pass A covers table rows < 32767, pass B covers the rest (rebased).
    Out-of-range slots get index -1, which the DGE turns into a read of
    the row *before* the pass base; the shipped table therefore carries
    explicit zero guard rows at both bases (and the tail), making every
    invalid-slot read a zero row that adds nothing. The hardware also
    skips any trailing run of -1s entirely (leaving stale SBUF data), so
    the last index of every gather is forced valid, pointing at a guard
    row when the real row belongs to the other pass.
  - The sum over the 32 active features runs on the tensor engine:
    identity matmuls accumulate every gathered tile into PSUM (fp32).
  - Epilogue on DVE/ACT: +b_ft, clip(0,1), dot with W_out, +b_out, sigmoid.
"""

import sys

sys.path.insert(0, "/opt/trn_rl_repo")

import numpy as np
import ml_dtypes

from concourse import bacc, bass, mybir
from concourse.masks import make_identity
import concourse.tile as tile
from concourse.bass_utils import run_bass_kernel_spmd

P = 128          # SBUF partitions
K = 32           # nnz (active features per position)
J = 8            # batch slots per partition
F = 512          # feature-table output width
NCORES = 8
BPC = P * J      # batch rows per core (1024)
FT_IN = 40960
ZPAD = 4096      # zero rows in front of each pass base (junk reads spread
                 # across them instead of hammering one HBM row)
SA = 32768 - ZPAD          # rows < SA -> pass A; rest -> pass B
# device table layout: [ZA(ZPAD), W[0:SA], ZB(ZPAD), W[SA:]]
VDEV = FT_IN + 2 * ZPAD
BOFF = ZPAD + SA           # byte row where the B view starts (= 32768)
S16 = BPC // 16  # columns of the 16-partition-wrapped index tile (64)

f32 = mybir.dt.float32
bf16 = mybir.dt.bfloat16
i16 = mybir.dt.int16
Alu = mybir.AluOpType

GBUFS = 8        # in-flight gather tiles per (A/B) tag
NQ = 4           # SWDGE descriptor-generation queues (parallel on HW)


def _build(fast: bool):
    nc = bacc.Bacc("TRN2", target_bir_lowering=False, debug=False, num_devices=NCORES,
                   num_swdge_queues=NQ)

    idx_in = {}
    for side in ("stm", "nstm"):
        for part in ("a", "b"):
            idx_in[(side, part)] = nc.dram_tensor(
                f"i{part}_{side}", [P, K, S16], i16, kind="ExternalInput"
            )
    wft = nc.dram_tensor("w_ft", [VDEV, F], bf16, kind="ExternalInput")
    bft = nc.dram_tensor("bft", [P, F], f32, kind="ExternalInput")
    w1 = nc.dram_tensor("w1", [P, F], f32, kind="ExternalInput")
    w2 = nc.dram_tensor("w2", [P, F], f32, kind="ExternalInput")
    bout = nc.dram_tensor("bout", [P, 1], f32, kind="ExternalInput")
    if not fast:
        vals = nc.dram_tensor("vals", [P, K, J], f32, kind="ExternalInput")
    out = nc.dram_tensor("out", [P, J], f32, kind="ExternalOutput")

    gbufs = GBUFS if fast else 2
    with tile.TileContext(nc) as tc:
        with tc.tile_pool(name="sbuf", bufs=1) as pool, \
             tc.tile_pool(name="gather", bufs=gbufs) as gpool, \
             tc.tile_pool(name="psum", bufs=1, space="PSUM") as ppool:
            idx_sb = {}
            for side_i, side in enumerate(("stm", "nstm")):
                for part in ("a", "b"):
                    t = pool.tile(
                        [P, K, S16], i16,
                        tag=f"i{part}{side_i}", name=f"i{part}_{side}_sb",
                    )
                    nc.sync.dma_start(out=t[:], in_=idx_in[(side, part)][:])
                    idx_sb[(side_i, part)] = t
            bft_sb = pool.tile([P, F], f32, tag="bft", name="bft_sb")
            nc.sync.dma_start(out=bft_sb[:], in_=bft[:])
            w_sb = [
                pool.tile([P, F], f32, tag="w1", name="w1_sb"),
                pool.tile([P, F], f32, tag="w2", name="w2_sb"),
            ]
            nc.sync.dma_start(out=w_sb[0][:], in_=w1[:])
            nc.sync.dma_start(out=w_sb[1][:], in_=w2[:])
            bout_sb = pool.tile([P, 1], f32, tag="bout", name="bout_sb")
            nc.sync.dma_start(out=bout_sb[:], in_=bout[:])
            ident = pool.tile([P, P], bf16, tag="ident", name="ident")
            make_identity(nc, ident[:])
            if not fast:
                vals_sb = pool.tile([P, K, J], f32, tag="vals", name="vals_sb")
                nc.sync.dma_start(out=vals_sb[:], in_=vals[:])

            def bcast(t2d):  # [P, F] -> [P, J, F] AP (stride-0 over J)
                return t2d[:].rearrange("p (j f) -> p j f", j=1).broadcast_to([P, J, F])

            z = [
                pool.tile([P, J], f32, tag=f"z{side}", name=f"z{side}")
                for side in range(2)
            ]
            for side in range(2):
                if fast:
                    acc = ppool.tile(
                        [P, J, F], f32, space="PSUM", tag="acc", name=f"acc{side}"
                    )
                else:
                    acc = pool.tile([P, J, F], f32, tag=f"sacc{side}", name=f"sacc{side}")
                for k in range(K):
                    ga = gpool.tile([P, J, F], bf16, tag="ga", name="ga")
                    gb = gpool.tile([P, J, F], bf16, tag="gb", name="gb")
                    qa = (side * 2 * K + 2 * k) % NQ
                    nc.gpsimd.dma_gather(
                        ga[:], wft[:, :], idx_sb[(side, "a")][:, k, :],
                        num_idxs=BPC, num_idxs_reg=BPC, elem_size=F,
                        queue_num=qa,
                    )
                    nc.gpsimd.dma_gather(
                        gb[:], wft[BOFF:, :], idx_sb[(side, "b")][:, k, :],
                        num_idxs=BPC, num_idxs_reg=BPC, elem_size=F,
                        queue_num=(qa + 1) % NQ,
                    )
                    if fast:
                        for j in range(J):
                            nc.tensor.matmul(
                                acc[:, j, :], ident[:], ga[:, j, :],
                                start=(k == 0), stop=False,
                            )
                        for j in range(J):
                            nc.tensor.matmul(
                                acc[:, j, :], ident[:], gb[:, j, :],
                                start=False, stop=(k == K - 1),
                            )
                    else:
                        vb = (
                            vals_sb[:, k, :]
                            .rearrange("p (j f) -> p j f", f=1)
                            .broadcast_to([P, J, F])
                        )
                        t = gpool.tile([P, J, F], f32, tag="t", name="t")
                        nc.vector.tensor_tensor(out=t[:], in0=ga[:], in1=gb[:], op=Alu.add)
                        if k == 0:
                            nc.vector.tensor_tensor(out=acc[:], in0=t[:], in1=vb, op=Alu.mult)
                        else:
                            nc.vector.tensor_tensor(out=t[:], in0=t[:], in1=vb, op=Alu.mult)
                            nc.vector.tensor_tensor(out=acc[:], in0=acc[:], in1=t[:], op=Alu.add)

                # epilogue: h = clip(acc + b_ft, 0, 1) * w_side; z = sum_f h
                h = pool.tile([P, J, F], f32, tag=f"h{side}", name=f"h{side}")
                nc.vector.tensor_tensor(out=h[:], in0=acc[:], in1=bcast(bft_sb), op=Alu.add)
                nc.vector.tensor_scalar(
                    out=h[:], in0=h[:], scalar1=0.0, scalar2=1.0,
                    op0=Alu.max, op1=Alu.min,
                )
                nc.vector.tensor_tensor(out=h[:], in0=h[:], in1=bcast(w_sb[side]), op=Alu.mult)
                nc.vector.tensor_reduce(
                    out=z[side][:], in_=h[:], axis=mybir.AxisListType.X, op=Alu.add
                )
            nc.vector.tensor_tensor(out=z[0][:], in0=z[0][:], in1=z[1][:], op=Alu.add)
            out_sb = pool.tile([P, J], f32, tag="out", name="out_sb")
            nc.scalar.activation(
                out=out_sb[:],
                in_=z[0][:],
                func=mybir.ActivationFunctionType.Sigmoid,
                bias=bout_sb[:, :1],
            )
            nc.sync.dma_start(out=out.ap(), in_=out_sb[:])

    nc.compile()
    return nc


_cache = {}


def _get(fast: bool):
    if fast not in _cache:
        _cache[fast] = _build(fast)
    return _cache[fast]


def _prep_table(W_ft: np.ndarray) -> np.ndarray:
    """f32 [40960, 512] -> bf16 [VDEV, 512]: zero pad blocks ahead of each
    pass segment so junk reads land on spread-out zero rows."""
    w = np.zeros((VDEV, F), dtype=ml_dtypes.bfloat16)
    w[ZPAD:ZPAD + SA] = W_ft[:SA].astype(ml_dtypes.bfloat16)
    w[BOFF + ZPAD:] = W_ft[SA:].astype(ml_dtypes.bfloat16)
    return w


def _prep_idx(idx_core: np.ndarray):
    """[1024, 32] int32 -> (A, B) int16 arrays of shape [128, 32, 64].

    Index g (= batch row b) for feature-slot k lives at partition g%16,
    column g//16 (replicated across the 8 16-partition groups).
    Out-of-pass slots read a (spread) zero row from the pass's ZPAD
    block, so every index is valid and every slot is written.
    """
    t3 = idx_core.astype(np.int64).reshape(S16, 16, K).transpose(2, 1, 0)  # [K,16,S16]
    spread = (np.arange(t3.size, dtype=np.int64).reshape(t3.shape) * 37) % ZPAD
    a = np.where(t3 < SA, t3 + ZPAD, spread).astype(np.int16)
    b = np.where(t3 >= SA, t3 - SA + ZPAD, spread).astype(np.int16)
    a = np.ascontiguousarray(np.tile(a, (1, 8, 1)).transpose(1, 0, 2))  # [128,K,S16]
    b = np.ascontiguousarray(np.tile(b, (1, 8, 1)).transpose(1, 0, 2))
    return a, b


def kernel(stm_indices, nstm_indices, values, W_ft, b_ft, W_out, b_out, _trace=False):
    stm_indices = np.asarray(stm_indices)
    nstm_indices = np.asarray(nstm_indices)
    values = np.asarray(values, dtype=np.float32)
    W_ft = np.ascontiguousarray(np.asarray(W_ft, dtype=np.float32))
    b_ft = np.asarray(b_ft, dtype=np.float32)
    W_out = np.asarray(W_out, dtype=np.float32)
    b_out = np.asarray(b_out, dtype=np.float32)

    fast = bool(np.all(values == 1.0))
    nc = _get(fast)

    w_dev = _prep_table(W_ft)
    bft_rep = np.ascontiguousarray(np.broadcast_to(b_ft, (P, F)).astype(np.float32))
    w1_rep = np.ascontiguousarray(np.broadcast_to(W_out[:F, 0], (P, F)).astype(np.float32))
    w2_rep = np.ascontiguousarray(np.broadcast_to(W_out[F:, 0], (P, F)).astype(np.float32))
    bout_rep = np.full((P, 1), b_out[0], dtype=np.float32)

    in_maps = []
    for c in range(NCORES):
        sl = slice(c * BPC, (c + 1) * BPC)
        m = {
            "w_ft": w_dev,
            "bft": bft_rep,
            "w1": w1_rep,
            "w2": w2_rep,
            "bout": bout_rep,
        }
        for side, arr in (("stm", stm_indices), ("nstm", nstm_indices)):
            a, b = _prep_idx(arr[sl])
            m[f"ia_{side}"] = a
            m[f"ib_{side}"] = b
        if not fast:
            # vals[p, k, j] = values[j*128 + p, k]
            m["vals"] = np.ascontiguousarray(
                values[sl].reshape(J, P, K).transpose(1, 2, 0)
            )
        in_maps.append(m)

    res = run_bass_kernel_spmd(
        nc, in_maps, core_ids=list(range(NCORES)), trace=_trace
    )
    # out[p, j] holds batch row j*128 + p
    out = np.concatenate(
        [res.results[c]["out"].T.reshape(BPC) for c in range(NCORES)]
    ).reshape(8192, 1)
    if _trace:
        return out, res
    return out


# revision 16
# speedup vs baseline: 4.0739x; 1.1277x over previous
"""Trainium2 Bass kernel for nn_NnBoard768 (sparse embedding lookup NNUE head).

Strategy (data-parallel over batch, 8 cores):
  - Each core handles 1024 of the 8192 batch rows. Batch row b sits at
    SBUF partition b%128, free-slot b//128.
  - The feature table is shipped to HBM as bf16 (half the gather traffic;
    PSUM accumulation below stays fp32, so only the table entries round).
  - Rows are fetched with the TIE-accelerated `dma_gather` instruction.
    Its indices are int16, so each (side, k) gather runs as two passes:
    pass A covers table rows < 32767, pass B covers the rest (rebased).
    Out-of-range slots get index -1, which the DGE turns into a read of
    the row *before* the pass base; the shipped table therefore carries
    explicit zero guard rows at both bases (and the tail), making every
    invalid-slot read a zero row that adds nothing. The hardware also
    skips any trailing run of -1s entirely (leaving stale SBUF data), so
    the last index of every gather is forced valid, pointing at a guard
    row when the real row belongs to the other pass.
  - The sum over the 32 active features runs on the tensor engine:
    identity matmuls accumulate every gathered tile into PSUM (fp32).
  - Epilogue on DVE/ACT: +b_ft, clip(0,1), dot with W_out, +b_out, sigmoid.
"""

import sys

sys.path.insert(0, "/opt/trn_rl_repo")

import numpy as np
import ml_dtypes

from concourse import bacc, bass, mybir
from concourse.masks import make_identity
import concourse.tile as tile
from concourse.bass_utils import run_bass_kernel_spmd

P = 128          # SBUF partitions
K = 32           # nnz (active features per position)
J = 8            # batch slots per partition
F = 512          # feature-table output width
NCORES = 8
BPC = P * J      # batch rows per core (1024)
FT_IN = 40960
ZPAD = 4096      # zero rows in front of each pass base (junk reads spread
                 # across them instead of hammering one HBM row)
SA = 32768 - ZPAD          # rows < SA -> pass A; rest -> pass B
# device table layout: [ZA(ZPAD), W[0:SA], ZB(ZPAD), W[SA:]]
VDEV = FT_IN + 2 * ZPAD
BOFF = ZPAD + SA           # byte row where the B view starts (= 32768)
S16 = BPC // 16  # columns of the 16-partition-wrapped index tile (64)

f32 = mybir.dt.float32
bf16 = mybir.dt.bfloat16
i16 = mybir.dt.int16
Alu = mybir.AluOpType

TDT = mybir.dt.float8e4            # gathered-table dtype on device
TDT_NP = ml_dtypes.float8_e4m3     # host equivalent
TSCALE = 64.0                      # host premultiplier; PE identity = 1/TSCALE

GBUFS = 8        # in-flight gather tiles per (A/B) tag
NQ = 4           # SWDGE descriptor-generation queues (parallel on HW)


def _build(fast: bool):
    nc = bacc.Bacc("TRN2", target_bir_lowering=False, debug=False, num_devices=NCORES,
                   num_swdge_queues=NQ)

    idx_in = {}
    for side in ("stm", "nstm"):
        for part in ("a", "b"):
            idx_in[(side, part)] = nc.dram_tensor(
                f"i{part}_{side}", [P, K, S16], i16, kind="ExternalInput"
            )
    wft = nc.dram_tensor("w_ft", [VDEV, F], TDT, kind="ExternalInput")
    bft = nc.dram_tensor("bft", [P, F], f32, kind="ExternalInput")
    w1 = nc.dram_tensor("w1", [P, F], f32, kind="ExternalInput")
    w2 = nc.dram_tensor("w2", [P, F], f32, kind="ExternalInput")
    bout = nc.dram_tensor("bout", [P, 1], f32, kind="ExternalInput")
    if not fast:
        vals = nc.dram_tensor("vals", [P, K, J], f32, kind="ExternalInput")
    out = nc.dram_tensor("out", [P, J], f32, kind="ExternalOutput")

    gbufs = GBUFS if fast else 2
    with tile.TileContext(nc) as tc:
        with tc.tile_pool(name="sbuf", bufs=1) as pool, \
             tc.tile_pool(name="gather", bufs=gbufs) as gpool, \
             tc.tile_pool(name="psum", bufs=1, space="PSUM") as ppool:
            idx_sb = {}
            for side_i, side in enumerate(("stm", "nstm")):
                for part in ("a", "b"):
                    t = pool.tile(
                        [P, K, S16], i16,
                        tag=f"i{part}{side_i}", name=f"i{part}_{side}_sb",
                    )
                    nc.sync.dma_start(out=t[:], in_=idx_in[(side, part)][:])
                    idx_sb[(side_i, part)] = t
            bft_sb = pool.tile([P, F], f32, tag="bft", name="bft_sb")
            nc.sync.dma_start(out=bft_sb[:], in_=bft[:])
            w_sb = [
                pool.tile([P, F], f32, tag="w1", name="w1_sb"),
                pool.tile([P, F], f32, tag="w2", name="w2_sb"),
            ]
            nc.sync.dma_start(out=w_sb[0][:], in_=w1[:])
            nc.sync.dma_start(out=w_sb[1][:], in_=w2[:])
            bout_sb = pool.tile([P, 1], f32, tag="bout", name="bout_sb")
            nc.sync.dma_start(out=bout_sb[:], in_=bout[:])
            ident = pool.tile([P, P], TDT, tag="ident", name="ident")
            make_identity(nc, ident[:])
            nc.vector.tensor_scalar_mul(ident[:], ident[:], 1.0 / TSCALE)
            if not fast:
                vals_sb = pool.tile([P, K, J], f32, tag="vals", name="vals_sb")
                nc.sync.dma_start(out=vals_sb[:], in_=vals[:])

            def bcast(t2d):  # [P, F] -> [P, J, F] AP (stride-0 over J)
                return t2d[:].rearrange("p (j f) -> p j f", j=1).broadcast_to([P, J, F])

            z = [
                pool.tile([P, J], f32, tag=f"z{side}", name=f"z{side}")
                for side in range(2)
            ]
            for side in range(2):
                if fast:
                    acc = ppool.tile(
                        [P, J, F], f32, space="PSUM", tag="acc", name=f"acc{side}"
                    )
                else:
                    acc = pool.tile([P, J, F], f32, tag=f"sacc{side}", name=f"sacc{side}")
                for k in range(K):
                    ga = gpool.tile([P, J, F], TDT, tag="ga", name="ga")
                    gb = gpool.tile([P, J, F], TDT, tag="gb", name="gb")
                    qa = (side * 2 * K + 2 * k) % NQ
                    nc.gpsimd.dma_gather(
                        ga[:], wft[:, :], idx_sb[(side, "a")][:, k, :],
                        num_idxs=BPC, num_idxs_reg=BPC, elem_size=F,
                        queue_num=qa,
                    )
                    nc.gpsimd.dma_gather(
                        gb[:], wft[BOFF:, :], idx_sb[(side, "b")][:, k, :],
                        num_idxs=BPC, num_idxs_reg=BPC, elem_size=F,
                        queue_num=(qa + 1) % NQ,
                    )
                    if fast:
                        for j in range(J):
                            nc.tensor.matmul(
                                acc[:, j, :], ident[:], ga[:, j, :],
                                start=(k == 0), stop=False,
                            )
                        for j in range(J):
                            nc.tensor.matmul(
                                acc[:, j, :], ident[:], gb[:, j, :],
                                start=False, stop=(k == K - 1),
                            )
                    else:
                        vb = (
                            vals_sb[:, k, :]
                            .rearrange("p (j f) -> p j f", f=1)
                            .broadcast_to([P, J, F])
                        )
                        t = gpool.tile([P, J, F], f32, tag="t", name="t")
                        nc.vector.tensor_tensor(out=t[:], in0=ga[:], in1=gb[:], op=Alu.add)
                        if k == 0:
                            nc.vector.tensor_tensor(out=acc[:], in0=t[:], in1=vb, op=Alu.mult)
                        else:
                            nc.vector.tensor_tensor(out=t[:], in0=t[:], in1=vb, op=Alu.mult)
                            nc.vector.tensor_tensor(out=acc[:], in0=acc[:], in1=t[:], op=Alu.add)

                # epilogue: h = clip(acc + b_ft, 0, 1) * w_side; z = sum_f h
                h = pool.tile([P, J, F], f32, tag=f"h{side}", name=f"h{side}")
                nc.vector.tensor_tensor(out=h[:], in0=acc[:], in1=bcast(bft_sb), op=Alu.add)
                nc.vector.tensor_scalar(
                    out=h[:], in0=h[:], scalar1=0.0, scalar2=1.0,
                    op0=Alu.max, op1=Alu.min,
                )
                nc.vector.tensor_tensor(out=h[:], in0=h[:], in1=bcast(w_sb[side]), op=Alu.mult)
                nc.vector.tensor_reduce(
                    out=z[side][:], in_=h[:], axis=mybir.AxisListType.X, op=Alu.add
                )
            nc.vector.tensor_tensor(out=z[0][:], in0=z[0][:], in1=z[1][:], op=Alu.add)
            out_sb = pool.tile([P, J], f32, tag="out", name="out_sb")
            nc.scalar.activation(
                out=out_sb[:],
                in_=z[0][:],
                func=mybir.ActivationFunctionType.Sigmoid,
                bias=bout_sb[:, :1],
            )
            nc.sync.dma_start(out=out.ap(), in_=out_sb[:])

    nc.compile()
    return nc


_cache = {}


def _get(fast: bool):
    if fast not in _cache:
        _cache[fast] = _build(fast)
    return _cache[fast]


def _prep_table(W_ft: np.ndarray) -> np.ndarray:
    """f32 [40960, 512] -> TSCALE-premultiplied TDT [VDEV, 512]: zero pad
    blocks ahead of each pass segment so junk reads land on spread-out
    zero rows."""
    w = np.zeros((VDEV, F), dtype=TDT_NP)
    w[ZPAD:ZPAD + SA] = (W_ft[:SA] * TSCALE).astype(TDT_NP)
    w[BOFF + ZPAD:] = (W_ft[SA:] * TSCALE).astype(TDT_NP)
    return w


def _prep_idx(idx_core: np.ndarray):
    """[1024, 32] int32 -> (A, B) int16 arrays of shape [128, 32, 64].

    Index g (= batch row b) for feature-slot k lives at partition g%16,
    column g//16 (replicated across the 8 16-partition groups).
    Out-of-pass slots read a (spread) zero row from the pass's ZPAD
    block, so every index is valid and every slot is written.
    """
    t3 = idx_core.astype(np.int64).reshape(S16, 16, K).transpose(2, 1, 0)  # [K,16,S16]
    spread = (np.arange(t3.size, dtype=np.int64).reshape(t3.shape) * 37) % ZPAD
    a = np.where(t3 < SA, t3 + ZPAD, spread).astype(np.int16)
    b = np.where(t3 >= SA, t3 - SA + ZPAD, spread).astype(np.int16)
    a = np.ascontiguousarray(np.tile(a, (1, 8, 1)).transpose(1, 0, 2))  # [128,K,S16]
    b = np.ascontiguousarray(np.tile(b, (1, 8, 1)).transpose(1, 0, 2))
    return a, b


def kernel(stm_indices, nstm_indices, values, W_ft, b_ft, W_out, b_out, _trace=False):
    stm_indices = np.asarray(stm_indices)
    nstm_indices = np.asarray(nstm_indices)
    values = np.asarray(values, dtype=np.float32)
    W_ft = np.ascontiguousarray(np.asarray(W_ft, dtype=np.float32))
    b_ft = np.asarray(b_ft, dtype=np.float32)
    W_out = np.asarray(W_out, dtype=np.float32)
    b_out = np.asarray(b_out, dtype=np.float32)

    fast = bool(np.all(values == 1.0))
    nc = _get(fast)

    w_dev = _prep_table(W_ft)
    bft_rep = np.ascontiguousarray(np.broadcast_to(b_ft, (P, F)).astype(np.float32))
    w1_rep = np.ascontiguousarray(np.broadcast_to(W_out[:F, 0], (P, F)).astype(np.float32))
    w2_rep = np.ascontiguousarray(np.broadcast_to(W_out[F:, 0], (P, F)).astype(np.float32))
    bout_rep = np.full((P, 1), b_out[0], dtype=np.float32)

    in_maps = []
    for c in range(NCORES):
        sl = slice(c * BPC, (c + 1) * BPC)
        m = {
            "w_ft": w_dev,
            "bft": bft_rep,
            "w1": w1_rep,
            "w2": w2_rep,
            "bout": bout_rep,
        }
        for side, arr in (("stm", stm_indices), ("nstm", nstm_indices)):
            a, b = _prep_idx(arr[sl])
            m[f"ia_{side}"] = a
            m[f"ib_{side}"] = b
        if not fast:
            # vals[p, k, j] = values[j*128 + p, k]
            m["vals"] = np.ascontiguousarray(
                values[sl].reshape(J, P, K).transpose(1, 2, 0) / TSCALE
            )
        in_maps.append(m)

    res = run_bass_kernel_spmd(
        nc, in_maps, core_ids=list(range(NCORES)), trace=_trace
    )
    # out[p, j] holds batch row j*128 + p
    out = np.concatenate(
        [res.results[c]["out"].T.reshape(BPC) for c in range(NCORES)]
    ).reshape(8192, 1)
    if _trace:
        return out, res
    return out
